# revision 26
# baseline (speedup 1.0000x reference)
"""Trainium2 Bass kernel for nn_MHA_75110388072824.

Multi-head attention, B=2, T=2048, D=2048, NH=16 heads (hd=128), fp32,
causal mask, y = softmax(mask((x Wq^T)(x Wk^T)^T / sqrt(hd))) (x Wv^T) Wo^T.

Sharding over 8 NeuronCores: core = b*4 + hg, b in {0,1} batch,
hg in {0..3} a group of 4 heads (tensor-parallel columns of Wq/Wk/Wv,
rows of Wo).  Each core computes a partial output [T, D] = Z_hg @ Wo_hg^T;
the host sums the 4 head-group partials per batch.

v2: bf16 end-to-end (validated 5.5e-3 rel err vs the 2e-2 gate).  QKV
stay resident in SBUF (no DRAM scratch round-trip), halving DMA traffic
vs v1.  bf16 matmuls run 1 cycle/row at any width, so causal diagonal
trimming at 128 granularity carries no narrow-tile penalty.  Softmax
denominator: off-diagonal exp strips are pre-added in quads on the DVE,
then one ones-matmul per quad (vs one per strip) cuts PE row-sum work.

Per-core pipeline:
  Phase A: stream x^T by 512-column chunks, weights resident; compute
           Q^T, K^T (per head, [hd=128 part, T]) and V ([t part, d]) into
           SBUF-resident bf16 tiles.
  Phase B: per head, per 512-wide t-chunk: S^T = K^T-block^T @ Q^T
           (causal blocks only), additive mask on diagonal blocks, exp on
           ACT -> bf16 strip, PV matmuls accumulate Z^T, quad-added
           strips -> ones-matmul row sums, normalize rows by 1/l.
  Phase C: out = Z^T^T @ Wo_hg^T via 4 head k-tiles, bf16 out to DRAM.
"""
import numpy as np

import concourse.bass as bass
import concourse.mybir as mybir
import concourse.tile as tile
from concourse import bacc
from concourse.bass_utils import run_bass_kernel_spmd

P = 128
T = 2048
D = 2048
NH = 16
HPC = 4            # heads per core
HD = 128
NT = T // P        # 16 t-blocks
NC4 = T // 512     # 4 512-chunks
KT = D // P        # 16 k-tiles over D
SCALE = 1.0 / float(np.sqrt(HD))
NEG = np.float32(-3.0e38)
F32 = mybir.dt.float32
BF16 = mybir.dt.bfloat16
EXP = mybir.ActivationFunctionType.Exp
ADD = mybir.AluOpType.add
MULT = mybir.AluOpType.mult

CFG = dict(xs=2, aps=8, strip=10, sps=5, lps=1, zps=2, lr=4, cev=3,
           cps=8, pd=4, mpre=False)


def _phase_a(tc, nc, xT, wqT, wkT, wvT, qk_sb, v_sb):
    # ---------------- Phase A: QKV projections -> SBUF-resident bf16.
    # Psum groups split by weight k-chunk so PE work starts as soon as the
    # first weight chunk lands.
    with (
        tc.tile_pool(name="wqkv", bufs=1) as wpool,
        tc.tile_pool(name="xs", bufs=CFG["xs"]) as xpool,
        tc.tile_pool(name="aps", bufs=CFG["aps"], space="PSUM") as aps,
    ):
        xTr = xT.rearrange("(ko p) t -> p ko t", p=P)
        wqTr = wqT.rearrange("(ko p) d -> p ko d", p=P)
        wkTr = wkT.rearrange("(ko p) d -> p ko d", p=P)
        wvTr = wvT.rearrange("(ko p) d -> p ko d", p=P)

        # Tiles are split so dependency granularity matches arrival order:
        # wq/xs0 get a tiny k=0 slice so the very first matmul starts as
        # soon as ~256KB lands, then 4-ktile pieces follow in consumption
        # order.  Separate tiles (not sub-slices of one tile) keep the
        # dependency tracking per-piece.
        def parts(pool, name, splits):
            out = []
            for a, b in splits:
                t = pool.tile([P, b - a, 512], BF16, tag=f"{name}{a}",
                              name=f"{name}{a}")
                out.append((a, b, t))
            return out

        SPL5 = [(0, 1), (1, 4), (4, 8), (8, 12), (12, 16)]
        SPL4 = [(0, 4), (4, 8), (8, 12), (12, 16)]
        wq_p = parts(wpool, "wq", SPL5)
        wk_p = parts(wpool, "wk", SPL4)
        wv_p = parts(wpool, "wv", SPL4)
        # xs0 parts live in the bufs=1 weight pool (used once, first chunk);
        # the rotating later chunks use the xs pool
        xs_parts = [parts(wpool, "xs0", SPL5)]

        def kmap(plist, k):
            for a, b, t in plist:
                if a <= k < b:
                    return t, k - a
            raise KeyError(k)

        # issue order: k=0 pair first on two parallel queues, then the
        # rest in consumption order (sync: weights then xs1 then wo;
        # scalar: xs0 then xs2/xs3)
        for (a, b, xt), (wa, wb, wt) in zip(xs_parts[0], wq_p):
            nc.scalar.dma_start(xt[:], xTr[:, a:b, 0:512])
            nc.sync.dma_start(wt[:], wqTr[:, wa:wb])
        for a, b, t in wk_p:
            nc.sync.dma_start(t[:], wkTr[:, a:b])
        for a, b, t in wv_p:
            nc.sync.dma_start(t[:], wvTr[:, a:b])
        for tci in range(1, NC4):
            xs = xpool.tile([P, KT, 512], BF16, tag="xs", name=f"xs{tci}")
            xs_parts.append([(0, KT, xs)])
            eng = nc.sync if tci == 1 else nc.scalar
            for kc in range(4):
                eng.dma_start(xs[:, 4 * kc:4 * (kc + 1)],
                              xTr[:, 4 * kc:4 * (kc + 1),
                                  512 * tci:512 * (tci + 1)])

        for tci in range(NC4):
            xp = xs_parts[tci]
            # Q then K; within each, 4 psum groups accumulate over k-chunks
            # in arrival order.
            for w_p, off in ((wq_p, 0), (wk_p, T)):
                pss = [aps.tile([P, 512], F32, tag="ps", name=f"ps{i}")
                       for i in range(HPC)]
                for kc in range(4):
                    for h in range(HPC):
                        for k in range(4 * kc, 4 * kc + 4):
                            wt, wj = kmap(w_p, k)
                            xt, xj = kmap(xp, k)
                            nc.tensor.matmul(
                                pss[h][:], wt[:, wj, h * P:(h + 1) * P],
                                xt[:, xj, :],
                                start=(k == 0), stop=(k == KT - 1))
                for h in range(HPC):
                    nc.vector.tensor_copy(
                        qk_sb[h][:, off + 512 * tci:off + 512 * (tci + 1)],
                        pss[h][:])
            # V: psum [t part, d]; evict whole 512-wide (all 4 heads' d).
            pss = [aps.tile([P, 512], F32, tag="ps", name=f"psv{i}")
                   for i in range(HPC)]
            for kc in range(4):
                for sb in range(4):
                    for k in range(4 * kc, 4 * kc + 4):
                        wt, wj = kmap(wv_p, k)
                        xt, xj = kmap(xp, k)
                        nc.tensor.matmul(
                            pss[sb][:], xt[:, xj, sb * P:(sb + 1) * P],
                            wt[:, wj, :],
                            start=(k == 0), stop=(k == KT - 1))
            for sb in range(4):
                st = 4 * tci + sb
                nc.vector.tensor_copy(v_sb[:, st, :], pss[sb][:])


def _phase_b(tc, nc, mask_t, ones_bf, qk_sb, v_sb, zt_tiles):
    # ---------------- Phase B: attention per head.  ZT stays in SBUF.
    with (
        tc.tile_pool(name="strip", bufs=CFG["strip"]) as spool,
        tc.tile_pool(name="lr", bufs=CFG["lr"]) as lrpool,
        tc.tile_pool(name="sps", bufs=CFG["sps"], space="PSUM") as sps,
        tc.tile_pool(name="lps", bufs=CFG["lps"], space="PSUM") as lps,
        tc.tile_pool(name="zps", bufs=CFG["zps"], space="PSUM") as zps,
    ):
        pd = CFG["pd"]
        # flat strip schedule across all (h, tc2) groups so the S->exp->PV
        # pipeline never drains at group boundaries
        sched = []
        for h in range(HPC):
            for tc2 in range(4):
                ns = 4 * tc2 + 4
                for si in range(ns):
                    sched.append((h, tc2, si, ns))
        n_all = len(sched)
        strips = {}
        groups = {}   # (h, tc2) -> dict(lsum, ztp, li, lacc)
        for ii in range(n_all + pd):
            if ii < n_all:
                h, tc2, si, ns = sched[ii]
                qk = qk_sb[h]
                q = si - 4 * tc2
                t0 = max(0, P * q)   # left edge of valid t range
                sp = sps.tile([P, 512], F32, tag="sp")
                if q >= 0 and CFG["mpre"]:
                    # preload the triangle mask into the psum bank; the S
                    # matmul then accumulates onto it (start=False), taking
                    # the mask add off the S->exp->PV critical chain
                    nc.vector.tensor_copy(sp[:, t0:], mask_t[:, q, t0:])
                nc.tensor.matmul(
                    sp[:, t0:], qk[:, T + si * P:T + (si + 1) * P],
                    qk[:, 512 * tc2 + t0:512 * (tc2 + 1)],
                    start=(q < 0 or not CFG["mpre"]), stop=True,
                    skip_group_check=True)
                if q >= 0 and not CFG["mpre"]:
                    # only the 128-wide diagonal sub-block needs the
                    # triangle mask; columns past it are fully valid
                    nc.vector.tensor_tensor(
                        sp[:, t0:t0 + P], sp[:, t0:t0 + P],
                        mask_t[:, q, t0:t0 + P], ADD)
                strip = spool.tile([P, 512], BF16, tag="strip")
                nc.scalar.activation(strip[:, t0:], sp[:, t0:], EXP,
                                     bias=0.0, scale=SCALE)
                strips[ii] = strip
            jj = ii - pd
            if jj < 0:
                continue
            h, tc2, si, ns = sched[jj]
            q = si - 4 * tc2
            t0 = max(0, P * q)
            strip = strips.pop(jj)
            g = groups.get((h, tc2))
            if g is None:
                g = groups[(h, tc2)] = dict(
                    lsum=lps.tile([P, 512], F32, tag="lsum", name="lsum"),
                    ztp=zps.tile([P, 512], F32, tag="ztp", name="ztp"),
                    li=0, nl=tc2 + 4, lacc=None)
            nc.tensor.matmul(g["ztp"][:, t0:],
                             v_sb[:, si, h * P:(h + 1) * P],
                             strip[:, t0:],
                             start=(si == 0), stop=(si == ns - 1))
            if q < 0:
                # off-diagonal: accumulate quads on DVE, one ones-matmul
                # per completed quad
                if si % 4 == 0:
                    g["lacc"] = strip
                else:
                    nc.vector.tensor_tensor(g["lacc"][:], g["lacc"][:],
                                            strip[:], ADD)
                if si % 4 == 3:
                    nc.tensor.matmul(g["lsum"][:], ones_bf[:], g["lacc"][:],
                                     start=(g["li"] == 0),
                                     stop=(g["li"] == g["nl"] - 1))
                    g["li"] += 1
            else:
                nc.tensor.matmul(g["lsum"][:, t0:], ones_bf[:],
                                 strip[:, t0:],
                                 start=(g["li"] == 0),
                                 stop=(g["li"] == g["nl"] - 1))
                g["li"] += 1
            if si == ns - 1:
                rlb = lrpool.tile([P, 512], F32, tag="rlb")
                nc.vector.reciprocal(rlb[:], g["lsum"][:])
                with nc.allow_low_precision(reason="zt in bf16"):
                    nc.vector.tensor_tensor(
                        zt_tiles[h][:, 512 * tc2:512 * (tc2 + 1)],
                        g["ztp"][:], rlb[:], MULT)
                del groups[(h, tc2)]


def _phase_c(tc, nc, wo_s, zt_tiles, out):
    # ---------------- Phase C: output projection from SBUF ZT, bf16 out.
    with (
        tc.tile_pool(name="cev", bufs=CFG["cev"]) as cev,
        tc.tile_pool(name="cps", bufs=CFG["cps"], space="PSUM") as cps,
    ):
        for ti in range(NT):
            ev = cev.tile([P, T], BF16, tag="cev")
            # last row-block: per-chunk DMAs so the final transfer is small
            # and the end-of-kernel drain stays short
            split = ti == NT - 1
            for oc in range(4):
                ps = cps.tile([P, 512], F32, tag="cps")
                for h in range(HPC):
                    nc.tensor.matmul(
                        ps[:], zt_tiles[h][:, ti * P:(ti + 1) * P],
                        wo_s[:, h, 512 * oc:512 * (oc + 1)],
                        start=(h == 0), stop=(h == HPC - 1))
                with nc.allow_low_precision(reason="bf16 out"):
                    nc.vector.tensor_copy(ev[:, 512 * oc:512 * (oc + 1)],
                                          ps[:])
                if split:
                    nc.sync.dma_start(
                        out[ti * P:(ti + 1) * P, 512 * oc:512 * (oc + 1)],
                        ev[:, 512 * oc:512 * (oc + 1)])
            if not split:
                nc.sync.dma_start(out[ti * P:(ti + 1) * P, :], ev[:])


def build(repeat=1, phases="ABC"):
    nc = bacc.Bacc("TRN2", target_bir_lowering=False, debug=False)
    xT = nc.dram_tensor("xT", [D, T], BF16, kind="ExternalInput").ap()
    wqT = nc.dram_tensor("wqT", [D, 512], BF16, kind="ExternalInput").ap()
    wkT = nc.dram_tensor("wkT", [D, 512], BF16, kind="ExternalInput").ap()
    wvT = nc.dram_tensor("wvT", [D, 512], BF16, kind="ExternalInput").ap()
    woT = nc.dram_tensor("woT", [512, D], BF16, kind="ExternalInput").ap()
    masks = nc.dram_tensor("masks", [P, 4, 512], F32, kind="ExternalInput").ap()
    out = nc.dram_tensor("out", [T, D], BF16, kind="ExternalOutput").ap()

    def emit_all():
        with tc.tile_pool(name="persist", bufs=1) as pp:
            qk_sb = [pp.tile([P, 2 * T], BF16, tag=f"qk{h}", name=f"qk{h}")
                     for h in range(HPC)]
            v_sb = pp.tile([P, NT, 512], BF16, tag="v")
            zt_tiles = [pp.tile([P, T], BF16, tag=f"zt{h}", name=f"zt{h}")
                        for h in range(HPC)]
            wo_s = pp.tile([P, HPC, T], BF16, tag="wo")
            mask_t = pp.tile([P, 4, 512], F32, tag="mask")
            ones_bf = pp.tile([P, P], BF16, tag="ones")
            nc.gpsimd.dma_start(mask_t[:], masks)
            nc.vector.memset(ones_bf[:], 1.0)
            woTr = woT.rearrange("(ko p) d -> p ko d", p=P)
            if "A" in phases:
                _phase_a(tc, nc, xT, wqT, wkT, wvT, qk_sb, v_sb)
            # wo rides the sync FIFO behind phase A's weight traffic so it
            # cannot steal bandwidth from the critical wq/wk/x streams
            for h in range(HPC):
                nc.sync.dma_start(wo_s[:, h], woTr[:, h])
            if "B" in phases:
                _phase_b(tc, nc, mask_t, ones_bf, qk_sb, v_sb, zt_tiles)
            if "C" in phases:
                _phase_c(tc, nc, wo_s, zt_tiles, out)

    with tile.TileContext(nc) as tc:
        if repeat == 1:
            emit_all()
        else:
            with tc.For_i(0, repeat, 1):
                emit_all()
    nc.compile()
    return nc


def make_inputs(x, Wq, Wk, Wv, Wo):
    """Host-side sharding: returns in_maps for cores 0..7 (core = b*4 + hg)."""
    import ml_dtypes
    bf = ml_dtypes.bfloat16
    # transposed mask: strip [s_local, q, t_local]; valid iff sl <= tl - 128*q
    masks = np.full((P, 4, 512), NEG, dtype=np.float32)
    for q in range(4):
        for sl in range(P):
            lo = sl + 128 * q
            if lo < 512:
                masks[sl, q, lo:] = 0.0
    xTs = [np.ascontiguousarray(x[b].T).astype(bf) for b in range(2)]
    in_maps = []
    for core in range(8):
        b, hg = core // 4, core % 4
        sl = slice(hg * 512, (hg + 1) * 512)
        in_maps.append({
            "xT": xTs[b],
            "wqT": np.ascontiguousarray(Wq[sl, :].T).astype(bf),
            "wkT": np.ascontiguousarray(Wk[sl, :].T).astype(bf),
            "wvT": np.ascontiguousarray(Wv[sl, :].T).astype(bf),
            "woT": np.ascontiguousarray(Wo[:, sl].T).astype(bf),
            "masks": masks,
        })
    return in_maps


_nc_cache = {}


def kernel(x, Wq, Wk, Wv, Wo):
    x = np.asarray(x, dtype=np.float32)
    Wq = np.asarray(Wq, dtype=np.float32)
    Wk = np.asarray(Wk, dtype=np.float32)
    Wv = np.asarray(Wv, dtype=np.float32)
    Wo = np.asarray(Wo, dtype=np.float32)
    if "nc" not in _nc_cache:
        _nc_cache["nc"] = build()
    nc = _nc_cache["nc"]
    in_maps = make_inputs(x, Wq, Wk, Wv, Wo)
    res = run_bass_kernel_spmd(nc, in_maps, core_ids=list(range(8)))
    B = x.shape[0]
    out = np.zeros((B, T, D), dtype=np.float32)
    for core in range(8):
        b = core // 4
        out[b] += np.asarray(res.results[core]["out"], dtype=np.float32)
    return out
